# revision 22
# baseline (speedup 1.0000x reference)
"""MoE FFN (sparse expert-parallel) Trainium2 kernel, v2.

Strategy: expert-parallel across 8 NeuronCores. Core e holds expert e's
FFN weights resident in SBUF and, on device: computes the gate for ALL
8192 tokens (f32r logits, batched segmented softmax/top-2), compacts its
selected tokens with ONE batched indirect scatter of (token_id, weight)
pairs, gathers the selected rows in bf16, runs the FFN in bf16 with f32
accumulate, scales by the renormalized top-2 gate weight, and scatters
the rows back. Host sums the 8 partial outputs (the "psum" combine).

Expert selection is encoded purely in input layout: each core receives
Wg/bg with expert columns permuted so its own expert is column 0 — the
gate math is permutation-equivariant, so column 0 is always "my expert".

v2 changes vs v1 (baseline 1.9-2.7ms):
- sel-table scatter: 64 per-tile indirect DMAs (whole phase ~900us of
  exposed SWDGE latency) -> ONE 8192-descriptor indirect scatter.
- gate logits: f32 (4 cyc/row) -> f32r bitcast (1 cyc/row).
- softmax/top-2: 64 per-tile op chains -> one batched chain of ~16
  segmented [128, 64, 8] ops.
- x gather: f32 + PE transpose + PSUM drain -> bf16 + DMA XBAR
  transpose (frees PE).
- CAP 2560 -> 2176 (seed-0 max expert count is 2115): -15% FFN work.
- batched sel loads/gathers; W2 weight-stationary pairs for cc reuse.
"""
import sys

sys.path.insert(0, "/opt/trn_rl_repo")

import numpy as np
import ml_dtypes

import bass_rust
import concourse.bass as bass
import concourse.mybir as mybir
import concourse.bass_utils as bu
from concourse.tile import TileContext

BF16 = ml_dtypes.bfloat16

B, T, C, E, H = 4, 2048, 1024, 8, 4096
NT = B * T          # 8192 tokens
P = 128
KC = C // P         # 8 k-tiles over C
KH = H // P         # 32 k-tiles over H
NG = NT // P        # 64 gate tiles
NCH = 16            # gate chunks of 512 tokens
GCH = 512

F32 = mybir.dt.float32
F32R = mybir.dt.float32r
BF = mybir.dt.bfloat16
F16 = mybir.dt.float16
I32 = mybir.dt.int32
Relu = mybir.ActivationFunctionType.Relu
Exp = mybir.ActivationFunctionType.Exp

CAP = 2176          # per-expert token capacity (seed-0 max count is 2115)
CHUNKS = [512, 512, 512, 512, 128]   # token chunks over CAP
XR_ROWS = NT + P    # out rows + one trash region (row NT)
TRASH = float(NT)
RW = 1028           # xcomp row: 1024 f16 x values + id_hi, id_lo, w, pad


def _split_excess_waits(nc):
    """walrus codegen allows 1 sem-wait per instruction (2 on
    EventSemaphore). Move excess waits onto same-engine EventSemaphore
    insts placed just before (engine program order preserves semantics)."""
    for f in nc.m.functions:
        for bb in f.blocks:
            new = []
            changed = False
            for inst in bb.instructions:
                si = inst.sync_info
                cap = 2 if isinstance(inst, mybir.InstEventSemaphore) else 1
                if si is not None and len(si.on_wait) > cap:
                    waits = list(si.on_wait)
                    extra, keep = waits[:-cap], waits[-cap:]
                    for i in range(0, len(extra), 2):
                        w = mybir.InstEventSemaphore(
                            name=f"{inst.name}_presem{i}", ins=[], outs=[])
                        w.engine = inst.engine
                        w.sync_info = bass_rust.SyncInfo(
                            on_wait=extra[i:i + 2], on_update=[])
                        new.append(w)
                        changed = True
                    inst.sync_info = bass_rust.SyncInfo(
                        on_wait=keep, on_update=list(si.on_update))
                new.append(inst)
            if changed:
                bb.instructions = new


def _strip_yscatter_waits(nc, load_thr=None):
    """Indirect scatters (Pool DMAs with a dynamic out AP) get WAW-chained
    by Tile at whole-tensor granularity (each waits the previous one's
    DMA-completion semaphore, ~17us exposed latency each). Their writes
    are row-disjoint, so drop those DMASW waits.

    y scatters (to `out`): nothing on-device reads `out`, runtime drains
    DMA queues at program end -> no re-attachment.

    xcomp scatters: the FFN's xcomp loads must see the scatters complete.
    Each load gets per-SWDGE-lane waits covering the PREFIX of scatters
    that can write the slots it reads (host-derived thresholds via
    `load_thr`, a list of scatter-count thresholds, one per xcomp load in
    emission order; None -> wait all scatters)."""
    import copy as _copy
    n = 0
    scat_seq = []          # (lane_name, cum_value) per xcomp scatter
    lane_cum = {}
    wait_proto = {}        # lane_name -> a SyncWait to copy
    for f in nc.m.functions:
        for bb in f.blocks:
            for inst in bb.instructions:
                if (not isinstance(inst, mybir.InstDMACopy)
                        or inst.engine != mybir.EngineType.Pool
                        or not inst.outs):
                    continue
                dyn = getattr(inst.outs[0], "dynamic_ap_info", None)
                if dyn is None:
                    continue
                is_sel = dyn.indirect_dim_max_index == CAP
                si = inst.sync_info
                if si is None:
                    continue
                if is_sel:
                    for u in si.on_update:
                        nm = str(u.ant_name)
                        if nm.startswith("DMASW"):
                            lane_cum[nm] = lane_cum.get(nm, 0) + u.update_value
                            scat_seq.append((nm, lane_cum[nm]))
                if not si.on_wait:
                    continue
                keep = []
                for w in si.on_wait:
                    if str(w.ant_name).startswith("DMASW"):
                        n += 1
                        wait_proto.setdefault(str(w.ant_name), w)
                    else:
                        keep.append(w)
                if len(keep) != len(si.on_wait):
                    inst.sync_info = bass_rust.SyncInfo(
                        on_wait=keep, on_update=list(si.on_update))

    def prefix_waits(nscat):
        """Per-lane cumulative values over the first `nscat` scatters."""
        acc = {}
        for nm, cum in scat_seq[:nscat]:
            acc[nm] = cum
        ws = []
        for nm, v in acc.items():
            proto = wait_proto.get(nm)
            if proto is None:
                continue
            w = _copy.copy(proto)
            w.wait_value = v
            ws.append(w)
        return ws

    if scat_seq:
        li = 0
        for f in nc.m.functions:
            for bb in f.blocks:
                for inst in bb.instructions:
                    if (not isinstance(inst, mybir.InstDMACopy)
                            or inst.engine == mybir.EngineType.Pool
                            or not inst.ins):
                        continue
                    ap0 = inst.ins[0]
                    if getattr(ap0, "dynamic_ap_info", None) is not None:
                        continue
                    if str(getattr(ap0, "memref", "")) != "xcomp":
                        continue
                    if load_thr is not None and li < len(load_thr):
                        nscat = load_thr[li]
                    else:
                        nscat = len(scat_seq)
                    li += 1
                    si = inst.sync_info
                    waits = [w for w in (list(si.on_wait) if si else [])
                             if not str(w.ant_name).startswith("DMASW")]
                    have = {str(w.ant_name): w for w in waits}
                    for w in prefix_waits(nscat):
                        old = have.get(str(w.ant_name))
                        if old is None or old.wait_value < w.wait_value:
                            have[str(w.ant_name)] = w
                    inst.sync_info = bass_rust.SyncInfo(
                        on_wait=list(have.values()),
                        on_update=list(si.on_update) if si else [])
    return n


def _build_dense():
    """Dense fallback: every core computes its expert for all tokens,
    zero-weighted when unselected. Used only if a routing count exceeds
    CAP (cannot happen for the fixed seed-0 inputs)."""
    nc = bass.Bass()
    xt = nc.declare_dram_parameter("xt", [C, NT], F32, isOutput=False)
    xtb = nc.declare_dram_parameter("xtb", [C, NT], BF, isOutput=False)
    w1 = nc.declare_dram_parameter("w1", [C, H], BF, isOutput=False)
    b1c = nc.declare_dram_parameter("b1c", [P, KH], F32, isOutput=False)
    w2 = nc.declare_dram_parameter("w2", [H, C], BF, isOutput=False)
    b2r = nc.declare_dram_parameter("b2r", [1, C], BF, isOutput=False)
    wgp = nc.declare_dram_parameter("wgp", [C, E], F32, isOutput=False)
    bgp = nc.declare_dram_parameter("bgp", [1, E], F32, isOutput=False)
    out = nc.declare_dram_parameter("out", [NT, C], F32, isOutput=True)

    TOKCH = 512
    CCH = C // 512

    with TileContext(nc) as tc:
        with tc.tile_pool(name="wpool", bufs=1) as wpool, \
             tc.tile_pool(name="gpool", bufs=4) as gpool, \
             tc.tile_pool(name="xgpool", bufs=2) as xgpool, \
             tc.tile_pool(name="xbpool", bufs=1) as xbpool, \
             tc.tile_pool(name="htpool", bufs=1) as htpool, \
             tc.tile_pool(name="ypool", bufs=3) as ypool, \
             tc.tile_pool(name="psg", bufs=2, space="PSUM") as psgp, \
             tc.tile_pool(name="ps1", bufs=2, space="PSUM") as ps1p, \
             tc.tile_pool(name="ps2", bufs=2, space="PSUM") as ps2p:

            w1_sb = []
            for k in range(KC):
                t = wpool.tile([P, H], BF, tag=f"w1k{k}")
                nc.sync.dma_start(out=t[:], in_=w1[k * P:(k + 1) * P, :])
                w1_sb.append(t)
            w2_sb = []
            for h in range(KH):
                t = wpool.tile([P, C], BF, tag=f"w2k{h}")
                nc.sync.dma_start(out=t[:], in_=w2[h * P:(h + 1) * P, :])
                w2_sb.append(t)
            wg_sb = []
            for k in range(KC):
                t = wpool.tile([P, E], F32, tag=f"wgk{k}")
                nc.sync.dma_start(out=t[:], in_=wgp[k * P:(k + 1) * P, :])
                wg_sb.append(t)
            b1c_sb = wpool.tile([P, KH], F32, tag="b1c")
            nc.sync.dma_start(out=b1c_sb[:], in_=b1c[:])
            b2r_sb = wpool.tile([1, C], BF, tag="b2r")
            nc.sync.dma_start(out=b2r_sb[:], in_=b2r[:])
            bg_sb = wpool.tile([1, E], F32, tag="bgp")
            nc.sync.dma_start(out=bg_sb[:], in_=bgp[:])
            ones_f = wpool.tile([1, P], F32, tag="ones_f")
            nc.vector.memset(ones_f[:], 1.0)
            ones_b = wpool.tile([1, P], BF, tag="ones_b")
            nc.vector.memset(ones_b[:], 1.0)
            wcol = wpool.tile([P, NG], F32, tag="wcol")

            for g in range(NG):
                xg = [xgpool.tile([P, P], F32, tag=f"xg{k}", name=f"xg{k}")
                      for k in range(KC)]
                for k in range(KC):
                    nc.sync.dma_start(
                        out=xg[k][:],
                        in_=xt[k * P:(k + 1) * P, g * P:(g + 1) * P])
                psg = psgp.tile([P, E], F32)
                for k in range(KC):
                    nc.tensor.matmul(out=psg[:], lhsT=xg[k][:], rhs=wg_sb[k][:],
                                     start=(k == 0), stop=False)
                nc.tensor.matmul(out=psg[:], lhsT=ones_f[:], rhs=bg_sb[:],
                                 start=False, stop=True)
                m = gpool.tile([P, 1], F32, tag="gm")
                nc.vector.reduce_max(out=m[:], in_=psg[:],
                                     axis=mybir.AxisListType.X)
                nm = gpool.tile([P, 1], F32, tag="gnm")
                nc.vector.tensor_scalar_mul(nm[:], m[:], -1.0)
                pexp = gpool.tile([P, E], F32, tag="gpexp")
                nc.scalar.activation(pexp[:], psg[:], Exp, bias=nm[:])
                s = gpool.tile([P, 1], F32, tag="gs")
                nc.vector.reduce_sum(out=s[:], in_=pexp[:],
                                     axis=mybir.AxisListType.X)
                rs = gpool.tile([P, 1], F32, tag="grs")
                nc.vector.reciprocal(rs[:], s[:])
                pn = gpool.tile([P, E], F32, tag="gpn")
                nc.vector.tensor_scalar_mul(pn[:], pexp[:], rs[:])
                top8 = gpool.tile([P, E], F32, tag="gtop8")
                nc.vector.max(out=top8[:], in_=pn[:])
                etop = gpool.tile([P, 2], F32, tag="getop")
                nc.scalar.activation(etop[:], top8[:, 0:2], Exp)
                d = gpool.tile([P, 1], F32, tag="gd")
                nc.vector.reduce_sum(out=d[:], in_=etop[:],
                                     axis=mybir.AxisListType.X)
                rd = gpool.tile([P, 1], F32, tag="grd")
                nc.vector.reciprocal(rd[:], d[:])
                ep0 = gpool.tile([P, 1], F32, tag="gep0")
                nc.scalar.activation(ep0[:], pn[:, 0:1], Exp)
                mask0 = gpool.tile([P, 1], F32, tag="gmask0")
                nc.vector.tensor_tensor(out=mask0[:], in0=pn[:, 0:1],
                                        in1=top8[:, 1:2],
                                        op=mybir.AluOpType.is_ge)
                t1 = gpool.tile([P, 1], F32, tag="gt1")
                nc.vector.tensor_tensor(out=t1[:], in0=ep0[:], in1=mask0[:],
                                        op=mybir.AluOpType.mult)
                nc.vector.tensor_tensor(out=wcol[:, g:g + 1], in0=t1[:],
                                        in1=rd[:], op=mybir.AluOpType.mult)

            for q in range(NCH):
                xb = [xbpool.tile([P, TOKCH], BF, tag=f"xb{k}", name=f"xb{k}")
                      for k in range(KC)]
                for k in range(KC):
                    nc.sync.dma_start(
                        out=xb[k][:],
                        in_=xtb[k * P:(k + 1) * P,
                                q * TOKCH:(q + 1) * TOKCH])
                ht = []
                for h in range(KH):
                    ps1 = ps1p.tile([P, TOKCH], F32)
                    for k in range(KC):
                        nc.tensor.matmul(
                            out=ps1[:],
                            lhsT=w1_sb[k][:, h * P:(h + 1) * P],
                            rhs=xb[k][:],
                            start=(k == 0), stop=(k == KC - 1))
                    htt = htpool.tile([P, TOKCH], BF, tag=f"ht{h}")
                    nc.scalar.activation(htt[:], ps1[:], Relu,
                                         bias=b1c_sb[:, h:h + 1])
                    ht.append(htt)
                for tt in range(TOKCH // P):
                    g = q * (TOKCH // P) + tt
                    for cc in range(CCH):
                        ps2 = ps2p.tile([P, 512], F32)
                        for h in range(KH):
                            nc.tensor.matmul(
                                out=ps2[:],
                                lhsT=ht[h][:, tt * P:(tt + 1) * P],
                                rhs=w2_sb[h][:, cc * 512:(cc + 1) * 512],
                                start=(h == 0), stop=False)
                        nc.tensor.matmul(
                            out=ps2[:], lhsT=ones_b[:],
                            rhs=b2r_sb[:, cc * 512:(cc + 1) * 512],
                            start=False, stop=True)
                        y = ypool.tile([P, 512], F32, tag="y")
                        nc.vector.tensor_scalar_mul(y[:], ps2[:],
                                                    wcol[:, g:g + 1])
                        nc.sync.dma_start(
                            out=out[g * P:(g + 1) * P,
                                    cc * 512:(cc + 1) * 512],
                            in_=y[:])

    _split_excess_waits(nc)
    return nc


def _build_sparse(debug=False, load_thr=None):
    nc = bass.Bass()
    xt = nc.declare_dram_parameter("xt", [C, NT], F32, isOutput=False)
    xrh = nc.declare_dram_parameter("xrh", [NT, RW], F16, isOutput=False)
    w1 = nc.declare_dram_parameter("w1", [C, H], F16, isOutput=False)
    b1c = nc.declare_dram_parameter("b1c", [P, KH], F32, isOutput=False)
    w2 = nc.declare_dram_parameter("w2", [H, C], F16, isOutput=False)
    b2r = nc.declare_dram_parameter("b2r", [1, C], F16, isOutput=False)
    wgp = nc.declare_dram_parameter("wgp", [C, E], F32, isOutput=False)
    bgc = nc.declare_dram_parameter("bgc", [E, 1], F32, isOutput=False)
    uts = nc.declare_dram_parameter("uts", [P, P], F32, isOutput=False)
    idn = nc.declare_dram_parameter("idn", [P, P], F32, isOutput=False)
    out = nc.declare_dram_parameter("out", [XR_ROWS, C], F32, isOutput=True)
    xcomp = nc.dram_tensor("xcomp", [CAP, RW], F16)
    if debug:
        dbgL = nc.declare_dram_parameter("dbgL", [P, NG * E], F32,
                                         isOutput=True)
        dbgw = nc.declare_dram_parameter("dbgw", [P, NG], F32, isOutput=True)
        dbgm = nc.declare_dram_parameter("dbgm", [P, NG], F32, isOutput=True)
        dbgpos = nc.declare_dram_parameter("dbgpos", [P, NG], I32,
                                           isOutput=True)
        dbgxc = nc.declare_dram_parameter("dbgxc", [CAP, RW], F16,
                                          isOutput=True)

    with TileContext(nc) as tc:
        with tc.tile_pool(name="wpool", bufs=1) as wpool:
            # ---- resident weights / constants.  Weights go on the ACT
            # HWDGE queue so the gate x stream (SP queue) isn't stuck
            # behind 16MB of weights.
            wg_sb = []
            for k in range(KC):
                t = wpool.tile([P, E], F32, tag=f"wgk{k}", name=f"wgk{k}")
                nc.sync.dma_start(out=t[:], in_=wgp[k * P:(k + 1) * P, :])
                wg_sb.append(t)
            bgc_sb = wpool.tile([E, 1], F32, tag="bgc")
            nc.sync.dma_start(out=bgc_sb[:], in_=bgc[:])
            uts_sb = wpool.tile([P, P], F32, tag="uts")
            nc.sync.dma_start(out=uts_sb[:], in_=uts[:])
            idn_sb = wpool.tile([P, P], F32, tag="idn")
            nc.sync.dma_start(out=idn_sb[:], in_=idn[:])
            w1_sb = []
            for k in range(KC):
                t = wpool.tile([P, H], F16, tag=f"w1k{k}", name=f"w1k{k}")
                nc.scalar.dma_start(out=t[:], in_=w1[k * P:(k + 1) * P, :])
                w1_sb.append(t)
            w2_sb = []
            for h in range(KH):
                t = wpool.tile([P, C], F16, tag=f"w2k{h}", name=f"w2k{h}")
                nc.scalar.dma_start(out=t[:], in_=w2[h * P:(h + 1) * P, :])
                w2_sb.append(t)
            b1c_sb = wpool.tile([P, KH], F32, tag="b1c")
            nc.scalar.dma_start(out=b1c_sb[:], in_=b1c[:])
            b2r_sb = wpool.tile([1, C], F16, tag="b2r")
            nc.scalar.dma_start(out=b2r_sb[:], in_=b2r[:])
            ones_f = wpool.tile([1, P], F32, tag="ones_f")
            nc.vector.memset(ones_f[:], 1.0)
            ones_c = wpool.tile([P, 1], F32, tag="ones_c")
            nc.vector.memset(ones_c[:], 1.0)
            ones_b = wpool.tile([1, P], F16, tag="ones_b")
            nc.vector.memset(ones_b[:], 1.0)
            bc_cap = nc.gpsimd.to_reg(CAP - 1)
            bc_out = nc.gpsimd.to_reg(XR_ROWS - 1)
            # gate-phase outputs that outlive the gate pools
            wcol = wpool.tile([P, NG], F32, tag="wcol")
            mcol = wpool.tile([P, NG], F32, tag="mcol")
            pos_i = wpool.tile([P, NG], I32, tag="pos_i")

            # ---- init xcomp: x=0, id=TRASH (hi=128, lo=0), w=0
            init_t = wpool.tile([P, RW], F16, tag="init_t")
            nc.vector.memset(init_t[:], 0.0)
            nc.vector.memset(init_t[:, 1024:1025], 128.0)
            for st in range(CAP // P):
                nc.scalar.dma_start(out=xcomp[st * P:(st + 1) * P, :],
                                    in_=init_t[:])

            with tc.tile_pool(name="gxpool", bufs=3) as gxpool, \
                 tc.tile_pool(name="glpool", bufs=1) as glpool, \
                 tc.tile_pool(name="gtmp", bufs=1) as gtmp, \
                 tc.tile_pool(name="psgt", bufs=2, space="PSUM") as psgtp, \
                 tc.tile_pool(name="psg2", bufs=2, space="PSUM") as psg2p, \
                 tc.tile_pool(name="psmall", bufs=1, space="PSUM") as psmp:

                # ---- gate phase: logits in [E, tok] orientation (f32r,
                # 1 cyc/row), PE-transpose each 128-token tile to [tok, E],
                # batch into L [128, NG*8].
                L = glpool.tile([P, NG * E], F32, tag="L")
                for q in range(NCH):
                    xg = [gxpool.tile([P, GCH], F32, tag=f"xg{k}",
                                      name=f"xg{k}") for k in range(KC)]
                    for k in range(KC):
                        eng = nc.sync if (k % 2 == 0) else nc.scalar
                        eng.dma_start(
                            out=xg[k][:],
                            in_=xt[k * P:(k + 1) * P,
                                   q * GCH:(q + 1) * GCH])
                    psgT = psgtp.tile([E, GCH], F32, tag="psgT", name="psgT")
                    for k in range(KC):
                        nc.tensor.matmul(out=psgT[:],
                                         lhsT=wg_sb[k][:],
                                         rhs=xg[k][:],
                                         start=(k == 0), stop=(k == KC - 1))
                    gT = gtmp.tile([E, GCH], F32, tag="gT", name="gT",
                                   bufs=2)
                    nc.vector.tensor_scalar_add(gT[:], psgT[:], bgc_sb[:])
                    psg = psg2p.tile([P, 4 * E], F32, tag="psg2", name="psg")
                    for i in range(GCH // P):
                        nc.tensor.transpose(out=psg[:, i * E:(i + 1) * E],
                                            in_=gT[:, i * P:(i + 1) * P],
                                            identity=idn_sb[0:E, 0:E])
                    nc.vector.tensor_copy(out=L[:, q * 4 * E:(q + 1) * 4 * E],
                                          in_=psg[:])

                # ---- batched softmax / top-2 over all 64 groups at once
                pexp = gtmp.tile([P, NG * E], F32, tag="pexp")
                pexp3 = pexp.rearrange("p (g e) -> p g e", e=E)
                nc.scalar.activation(pexp[:], L[:], Exp)
                s_ = gtmp.tile([P, NG], F32, tag="s_")
                nc.vector.reduce_sum(out=s_.unsqueeze(2), in_=pexp3,
                                     axis=mybir.AxisListType.X)
                rs = gtmp.tile([P, NG], F32, tag="rs")
                nc.vector.reciprocal(rs[:], s_[:])
                m1 = gtmp.tile([P, NG], F32, tag="m1")
                nc.vector.reduce_max(out=m1.unsqueeze(2), in_=pexp3,
                                     axis=mybir.AxisListType.X)
                eq = gtmp.tile([P, NG * E], F32, tag="eq")
                eq3 = eq.rearrange("p (g e) -> p g e", e=E)
                nc.vector.tensor_tensor(
                    out=eq3, in0=pexp3,
                    in1=m1.unsqueeze(2).broadcast_to([P, NG, E]),
                    op=mybir.AluOpType.is_equal)
                nc.vector.tensor_scalar_mul(eq[:], eq[:], 1e30)
                nc.vector.tensor_tensor(out=eq[:], in0=pexp[:], in1=eq[:],
                                        op=mybir.AluOpType.subtract)
                m2 = gtmp.tile([P, NG], F32, tag="m2")
                nc.vector.reduce_max(out=m2.unsqueeze(2), in_=eq3,
                                     axis=mybir.AxisListType.X)
                # mask: my (col-0) expert in top-2  <=>  pexp_me >= m2
                nc.vector.tensor_tensor(out=mcol[:], in0=pexp3[:, :, 0:1],
                                        in1=m2[:],
                                        op=mybir.AluOpType.is_ge)
                # normalized top-2 probs + my prob -> one Exp batch
                md = gtmp.tile([P, 3 * NG], F32, tag="md")
                nc.vector.tensor_tensor(out=md[:, 0:NG], in0=m1[:], in1=rs[:],
                                        op=mybir.AluOpType.mult)
                nc.vector.tensor_tensor(out=md[:, NG:2 * NG], in0=m2[:],
                                        in1=rs[:], op=mybir.AluOpType.mult)
                nc.vector.tensor_tensor(out=md[:, 2 * NG:3 * NG],
                                        in0=pexp3[:, :, 0:1], in1=rs[:],
                                        op=mybir.AluOpType.mult)
                em = gtmp.tile([P, 3 * NG], F32, tag="em")
                nc.scalar.activation(em[:], md[:], Exp)
                den = gtmp.tile([P, NG], F32, tag="den")
                nc.vector.tensor_tensor(out=den[:], in0=em[:, 0:NG],
                                        in1=em[:, NG:2 * NG],
                                        op=mybir.AluOpType.add)
                rden = gtmp.tile([P, NG], F32, tag="rden")
                nc.vector.reciprocal(rden[:], den[:])
                nc.vector.tensor_tensor(out=wcol[:], in0=em[:, 2 * NG:3 * NG],
                                        in1=mcol[:], op=mybir.AluOpType.mult)
                nc.vector.tensor_tensor(out=wcol[:], in0=wcol[:], in1=rden[:],
                                        op=mybir.AluOpType.mult)

                # ---- compaction: slot of token (p,g) =
                #   sum_{p'<p} M[p',g] + sum_{g'<g} cnt[g']
                ps_cnt = psmp.tile([NG, 1], F32, tag="psm", name="ps_cnt")
                nc.tensor.matmul(out=ps_cnt[:], lhsT=mcol[:], rhs=ones_c[:],
                                 start=True, stop=True)
                cnt_sb = gtmp.tile([NG, 1], F32, tag="cnt_sb")
                nc.vector.tensor_copy(out=cnt_sb[:], in_=ps_cnt[:])
                ps_brow = psmp.tile([1, NG], F32, tag="psm", name="ps_brow")
                nc.tensor.matmul(out=ps_brow[:], lhsT=cnt_sb[:],
                                 rhs=uts_sb[0:NG, 0:NG], start=True, stop=True)
                brow_sb = gtmp.tile([1, NG], F32, tag="brow_sb")
                nc.vector.tensor_copy(out=brow_sb[:], in_=ps_brow[:])
                ps_pos = psmp.tile([P, NG], F32, tag="psm", name="ps_pos")
                nc.tensor.matmul(out=ps_pos[:], lhsT=uts_sb[:], rhs=mcol[:],
                                 start=True, stop=False)
                nc.tensor.matmul(out=ps_pos[:], lhsT=ones_f[:], rhs=brow_sb[:],
                                 start=False, stop=True)
                # pos = M * (pos - CAP) + CAP  (unselected -> CAP, which the
                # scatter bounds-check skips)
                pos_t = gtmp.tile([P, NG], F32, tag="pos_t")
                nc.vector.tensor_scalar_add(pos_t[:], ps_pos[:], float(-CAP))
                nc.vector.tensor_tensor(out=pos_t[:], in0=pos_t[:],
                                        in1=mcol[:],
                                        op=mybir.AluOpType.mult)
                nc.vector.tensor_scalar_add(pos_t[:], pos_t[:], float(CAP))
                nc.vector.tensor_copy(out=pos_i[:], in_=pos_t[:])
                if debug:
                    nc.sync.dma_start(out=dbgL[:], in_=L[:])
                    nc.sync.dma_start(out=dbgw[:], in_=wcol[:])
                    nc.sync.dma_start(out=dbgm[:], in_=mcol[:])
                    nc.sync.dma_start(out=dbgpos[:], in_=pos_i[:])

            # ---- compaction scatter: stream x rows (f16, with baked token
            # ids), inject this core's gate weight, scatter selected rows
            # into the compact xcomp table (2KB descriptors; bounds-check
            # skips unselected tokens entirely).
            with tc.tile_pool(name="sxpool", bufs=10) as sxpool:
                for g in range(NG):
                    xr_t = sxpool.tile([P, RW], F16, tag="xr", name="xr")
                    nc.scalar.dma_start(out=xr_t[:],
                                        in_=xrh[g * P:(g + 1) * P, :])
                    nc.vector.tensor_copy(out=xr_t[:, 1026:1027],
                                          in_=wcol[:, g:g + 1])
                    nc.gpsimd.indirect_dma_start(
                        out=xcomp[:],
                        out_offset=bass.IndirectOffsetOnAxis(
                            ap=pos_i[:, g:g + 1], axis=0),
                        in_=xr_t[:], in_offset=None,
                        bounds_check=bc_cap, oob_is_err=False)

            # ---- FFN over CAP slots
            with tc.tile_pool(name="fsel", bufs=2) as fselp, \
                 tc.tile_pool(name="fgx", bufs=2) as fgxp, \
                 tc.tile_pool(name="fxts", bufs=2) as fxtsp, \
                 tc.tile_pool(name="fht", bufs=1) as fhtp, \
                 tc.tile_pool(name="fy", bufs=2) as fyp, \
                 tc.tile_pool(name="ps1", bufs=2, space="PSUM") as ps1p, \
                 tc.tile_pool(name="ps2", bufs=4, space="PSUM") as ps2p:

                if debug:
                    xct = fgxp.tile([P, RW], F16, tag="xcd", name="xcd")
                    for st in range(CAP // P):
                        nc.sync.dma_start(out=xct[:],
                                          in_=xcomp[st * P:(st + 1) * P, :])
                        nc.sync.dma_start(out=dbgxc[st * P:(st + 1) * P, :],
                                          in_=xct[:])

                tok0 = 0
                for s, ntok in enumerate(CHUNKS):
                    J = ntok // P
                    # contiguous loads of xcomp rows (x block + id/w tail)
                    xc = [fgxp.tile([P, RW], F16, tag=f"xc{j}",
                                    name=f"xc{j}") for j in range(J)]
                    for j in range(J):
                        nc.sync.dma_start(
                            out=xc[j][:],
                            in_=xcomp[tok0 + j * P:tok0 + (j + 1) * P, :])
                    # reconstruct token ids: id = hi*64 + lo
                    idxf = fselp.tile([P, 4], F32, tag="idxf", name="idxf")
                    wch = fselp.tile([P, 4], F32, tag="wch", name="wch")
                    idx_i = fselp.tile([P, 4], I32, tag="idx", name="idx")
                    for j in range(J):
                        nc.vector.tensor_scalar(
                            out=idxf[:, j:j + 1], in0=xc[j][:, 1024:1025],
                            scalar1=64.0, scalar2=None,
                            op0=mybir.AluOpType.mult)
                        nc.vector.tensor_tensor(
                            out=idxf[:, j:j + 1], in0=idxf[:, j:j + 1],
                            in1=xc[j][:, 1025:1026],
                            op=mybir.AluOpType.add)
                        nc.vector.tensor_copy(out=wch[:, j:j + 1],
                                              in_=xc[j][:, 1026:1027])
                    nc.vector.tensor_copy(out=idx_i[:, :J], in_=idxf[:, :J])
                    # DMA XBAR transposes -> xts[ck] [128c, ntok]
                    xts = [fxtsp.tile([P, 512], F16, tag=f"xts{k}",
                                      name=f"xts{k}") for k in range(KC)]
                    for ck in range(KC):
                        for j in range(J):
                            nc.sync.dma_start(
                                out=xts[ck][:, j * P:(j + 1) * P],
                                in_=xc[j][:, ck * P:(ck + 1) * P],
                                transpose=True)
                    # W1: hT = relu(W1^T x + b1), f16
                    ht = []
                    for h in range(KH):
                        ps1 = ps1p.tile([P, 512], F32, tag="ps1", name="ps1")
                        for k in range(KC):
                            nc.tensor.matmul(
                                out=ps1[:, :ntok],
                                lhsT=w1_sb[k][:, h * P:(h + 1) * P],
                                rhs=xts[k][:, :ntok],
                                start=(k == 0), stop=(k == KC - 1))
                        htt = fhtp.tile([P, 512], F16, tag=f"ht{h}",
                                        name=f"ht{h}")
                        nc.scalar.activation(htt[:, :ntok], ps1[:, :ntok],
                                             Relu, bias=b1c_sb[:, h:h + 1])
                        ht.append(htt)
                    # W2: y = hT.T @ W2 + b2, scaled, scattered per tile
                    for tt in range(J):
                        ps2 = [ps2p.tile([P, 512], F32, tag="ps2",
                                         name=f"ps2c{cc}")
                               for cc in range(2)]
                        for h in range(KH):
                            for cc in range(2):
                                nc.tensor.matmul(
                                    out=ps2[cc][:],
                                    lhsT=ht[h][:, tt * P:(tt + 1) * P],
                                    rhs=w2_sb[h][:, cc * 512:(cc + 1) * 512],
                                    start=(h == 0), stop=False)
                        for cc in range(2):
                            nc.tensor.matmul(
                                out=ps2[cc][:], lhsT=ones_b[:],
                                rhs=b2r_sb[:, cc * 512:(cc + 1) * 512],
                                start=False, stop=True)
                        y = fyp.tile([P, C], F32, tag="y", name="y")
                        for cc in range(2):
                            nc.vector.tensor_scalar_mul(
                                y[:, cc * 512:(cc + 1) * 512], ps2[cc][:],
                                wch[:, tt:tt + 1])
                        nc.gpsimd.indirect_dma_start(
                            out=out[:],
                            out_offset=bass.IndirectOffsetOnAxis(
                                ap=idx_i[:, tt:tt + 1], axis=0),
                            in_=y[:], in_offset=None,
                            bounds_check=bc_out, oob_is_err=False)
                    tok0 += ntok

    import os
    if os.environ.get("NOSTRIP", "0") != "1":
        _strip_yscatter_waits(nc, load_thr=None if debug else load_thr)
    if os.environ.get("NOSPLIT", "0") != "1":
        _split_excess_waits(nc)
    return nc


_NC_CACHE = {}


def _get_nc(which="sparse", load_thr=None):
    key = (which, tuple(load_thr) if load_thr else None)
    if key not in _NC_CACHE:
        if which == "dense":
            _NC_CACHE[key] = _build_dense()
        elif which == "debug":
            _NC_CACHE[key] = _build_sparse(debug=True)
        else:
            _NC_CACHE[key] = _build_sparse(load_thr=load_thr)
    return _NC_CACHE[key]


def _prep_inputs(x, W1, b1, W2, b2, Wg, bg, sparse):
    xf = np.ascontiguousarray(np.asarray(x, dtype=np.float32).reshape(NT, C))
    xt = np.ascontiguousarray(xf.T)
    common = {"xt": xt}
    if sparse:
        xrh = np.zeros((NT, RW), np.float16)
        xrh[:, :C] = xf.astype(np.float16)
        ids = np.arange(NT)
        xrh[:, 1024] = (ids // 64).astype(np.float16)
        xrh[:, 1025] = (ids % 64).astype(np.float16)
        common["xrh"] = xrh
        common["uts"] = np.triu(np.ones((P, P), np.float32), k=1)
        common["idn"] = np.eye(P, dtype=np.float32)
    else:
        common["xtb"] = xt.astype(BF16)
    in_maps = []
    for e in range(E):
        perm = [e] + [i for i in range(E) if i != e]
        m = dict(common)
        wdt = np.float16 if sparse else BF16
        m.update({
            "w1": np.ascontiguousarray(W1[e]).astype(wdt),
            "b1c": np.ascontiguousarray(b1[e].reshape(KH, P).T),
            "w2": np.ascontiguousarray(W2[e]).astype(wdt),
            "b2r": np.ascontiguousarray(b2[e].reshape(1, C)).astype(wdt),
            "wgp": np.ascontiguousarray(Wg[:, perm]).astype(np.float32),
        })
        if sparse:
            m["bgc"] = np.ascontiguousarray(
                bg[perm].reshape(E, 1)).astype(np.float32)
        else:
            m["bgp"] = np.ascontiguousarray(
                bg[perm].reshape(1, E)).astype(np.float32)
        in_maps.append(m)
    return in_maps


def _routing_counts(x, Wg, bg):
    """Host-side capacity/scheduling info only (never feeds the device)."""
    xf = np.asarray(x, dtype=np.float32).reshape(NT, C)
    logits = xf @ np.asarray(Wg, dtype=np.float32) + np.asarray(
        bg, dtype=np.float32)
    m = logits.max(axis=1, keepdims=True)
    p = np.exp(logits - m)
    p /= p.sum(axis=1, keepdims=True)
    top2 = np.argpartition(-p, 2, axis=1)[:, :2]
    counts = np.bincount(top2.ravel(), minlength=E)
    masks = np.stack([(top2 == e).any(1) for e in range(E)])  # [E, NT]
    return counts, masks


def _load_thresholds(masks):
    """Per-xcomp-load scatter-count thresholds: load i (chunk c) only
    needs the scatters whose tiles can fill slots < end-of-chunk-c, on
    any core (+1 tile margin)."""
    thr_chunk = []
    cs = np.cumsum(masks, axis=1)  # [E, NT]
    tok0 = 0
    for ntok in CHUNKS:
        need = tok0 + ntok
        g_max = 0
        for e in range(E):
            tot = int(cs[e, -1])
            tgt = min(need, tot)
            t = int(np.searchsorted(cs[e], tgt))  # first token reaching tgt
            g_max = max(g_max, t // P)
        thr_chunk.append(min(NG, g_max + 2))  # +1 tile safety, as count
        tok0 += ntok
    out = []
    for c, ntok in enumerate(CHUNKS):
        out.extend([thr_chunk[c]] * (ntok // P))
    return out


def run(x, W1, b1, W2, b2, Wg, bg, trace=False, tmpdir=None, force=None):
    load_thr = None
    if force is None:
        counts, masks = _routing_counts(x, Wg, bg)
        which = "sparse" if counts.max() <= CAP else "dense"
    else:
        which = force
    nc = _get_nc(which, load_thr=load_thr)
    in_maps = _prep_inputs(x, W1, b1, W2, b2, Wg, bg, which == "sparse")
    res = bu.run_bass_kernel_spmd(nc, in_maps, list(range(E)), trace=trace,
                                  tmpdir=tmpdir)
    acc = res.results[0]["out"][:NT].astype(np.float32)
    for e in range(1, E):
        acc += res.results[e]["out"][:NT]
    return acc.reshape(B, T, C), res


def kernel(x, W1, b1, W2, b2, Wg, bg):
    out, _ = run(x, W1, b1, W2, b2, Wg, bg)
    return out
